# revision 11
# baseline (speedup 1.0000x reference)
"""Trainium2 kernel for per-row iterative 2-bit affine quantization (ALS).

Self-contained: accepts FULL inputs w[4096,8192] f32 + mask[4096,8192] bool,
shards rows across 8 NeuronCores (512 rows each), runs the 5-iteration ALS
on-device per row group, and returns the FULL [4096,8192] f32 output.

Per-core layout: 512 rows = 4 independent row-groups of 128 partitions; the
x shard (16 MiB) stays SBUF-resident. Each row-group carries its own ALS
state so the groups software-pipeline around the per-pass parameter update.

Stats-pass stages per 2048-wide chunk (emitted with explicit slot lags so
each engine's in-order stream never waits on the same chunk's earlier
stages):
  S1 (ACT):  w = relu(x*invs + nzp)          nzp = -o/s
  S2 (ts):   v = min(w + 2^23, 2^23 + 3)     exact RNE round + upper clamp
  S3 (DVE):  q16 = v - 2^23, accum -> sum(q); stt (x-o)*q16 accum -> numer
  S4 (ACT):  square(q16) -> PSUM, accum -> sum(q^2)
"""

import numpy as np

R_TOTAL, C = 4096, 8192
N_CORES = 8
R = R_TOTAL // N_CORES          # 512 rows per core
G = R // 128                    # 4 row-groups of 128 partitions
HF = 2048                       # chunk width (4 per group)
NH = C // HF                    # chunks per group
NCH = G * NH                    # chunks per pass (16)
MAXQ = 3.0
EPS = 1e-8
N_PASS = 5
MAGIC = 8388608.0               # 2^23: add/sub rounds non-negative f32 (RNE)
INF = 3.4e38

_CACHE = {}

# Engine per chunk index (len NCH) for tunable units: 'A'=ACT 'D'=DVE 'P'=Pool
CFG = {
    "ts_b": "PPPPPPPPPPPPPPPP",
    "den":  "AAAADAAAADAAAADA",
    "fin_b": "PPPPPPPPPPPPPPPP",
    "fin_c": "PPPPPPPPPPPPPPPP",
    "fin_r": "DDDDDDDDDDDDDDDD",
    "upd_pool": True,
}


def _build_program(n_devices=N_CORES, with_collective=False, cfg=None):
    import concourse.bacc as bacc
    import concourse.mybir as mybir
    from concourse import tile

    if cfg is None:
        cfg = CFG
    f32 = mybir.dt.float32
    f16 = mybir.dt.float16
    Alu = mybir.AluOpType
    Act = mybir.ActivationFunctionType

    nc = bacc.Bacc("TRN2", target_bir_lowering=False, debug=False,
                   num_devices=n_devices)
    x_d = nc.dram_tensor("x", [R, C], f32, kind="ExternalInput").ap()
    out_d = nc.dram_tensor("out", [R, C], f32, kind="ExternalOutput").ap()

    with tile.TileContext(nc) as tc:
        with (
            tc.tile_pool(name="big", bufs=1) as big,
            tc.tile_pool(name="wp", bufs=5) as wp,
            tc.tile_pool(name="qp", bufs=4) as qp,
            tc.tile_pool(name="sp", bufs=2) as sp,
            tc.tile_pool(name="ps", bufs=2, space="PSUM") as psp,
            tc.tile_pool(name="small", bufs=1) as small,
        ):
            xs = big.tile([128, G * C], f32, tag="xs", name="xs")

            def xsl(g, h):  # resident x view [128, HF]
                off = g * C + h * HF
                return xs[:, off:off + HF]

            def st(name, n=1):
                return small.tile([128, n], f32, tag=name, name=name)

            # per-group state
            vminc = [st(f"vmin{g}", NH) for g in range(G)]
            vmaxc = [st(f"vmax{g}", NH) for g in range(G)]
            sxc = [st(f"sxc{g}", NH) for g in range(G)]
            sxN = [st(f"sxN{g}") for g in range(G)]
            scur = [st(f"scur{g}") for g in range(G)]
            ocur = [st(f"ocur{g}") for g in range(G)]
            invs = [st(f"invs{g}") for g in range(G)]
            nzp = [st(f"nzp{g}") for g in range(G)]
            sqc = [st(f"sqc{g}", NH) for g in range(G)]
            numc = [st(f"numc{g}", NH) for g in range(G)]
            denc = [st(f"denc{g}", NH) for g in range(G)]
            t1 = [st(f"t1{g}") for g in range(G)]
            t2 = [st(f"t2{g}") for g in range(G)]
            t3 = [st(f"t3{g}") for g in range(G)]

            eng = {"D": nc.vector, "P": nc.gpsimd}
            X = mybir.AxisListType.X

            def comb(cols, out):  # [128, NH] -> [128, 1] sum
                nc.vector.tensor_reduce(
                    out[:], cols[:].rearrange("p (a b) -> p a b", a=1),
                    axis=X, op=Alu.add)

            def chunk(i):
                return i // NH, i % NH

            # ================= LOAD (pipelined) =================
            def load_s1(i):
                g, h = chunk(i)
                nc.sync.dma_start(
                    xsl(g, h),
                    x_d[g * 128:(g + 1) * 128, h * HF:(h + 1) * HF])

            def load_s2(i):
                g, h = chunk(i)
                scr = wp.tile([128, HF], f32, tag="w", name="w")
                nc.vector.tensor_scalar(
                    out=scr[:], in0=xsl(g, h), scalar1=INF, scalar2=None,
                    op0=Alu.min, op1=Alu.min, accum_out=vminc[g][:, h:h + 1])

            def load_s3(i):
                g, h = chunk(i)
                scr = wp.tile([128, HF], f32, tag="w", name="w")
                nc.vector.tensor_scalar(
                    out=scr[:], in0=xsl(g, h), scalar1=-INF, scalar2=None,
                    op0=Alu.max, op1=Alu.max, accum_out=vmaxc[g][:, h:h + 1])

            def load_s4(i):
                g, h = chunk(i)
                pscr = psp.tile([128, HF], f32, tag="ps", name="ps")
                nc.scalar.activation(
                    out=pscr[:], in_=xsl(g, h), func=Act.Identity,
                    bias=0.0, scale=1.0, accum_out=sxc[g][:, h:h + 1])

            def init_group(g):
                va, vb, vc = t1[g], t2[g], t3[g]
                nc.vector.tensor_reduce(
                    va[:], vminc[g][:].rearrange("p (a b) -> p a b", a=1),
                    axis=X, op=Alu.min)
                nc.vector.tensor_reduce(
                    vb[:], vmaxc[g][:].rearrange("p (a b) -> p a b", a=1),
                    axis=X, op=Alu.max)
                nc.vector.tensor_scalar(out=va[:], in0=va[:], scalar1=0.0,
                                        scalar2=None, op0=Alu.min)  # xmin
                nc.vector.tensor_scalar(out=vb[:], in0=vb[:], scalar1=0.0,
                                        scalar2=None, op0=Alu.max)  # xmax
                nc.vector.tensor_sub(vb[:], vb[:], va[:])
                nc.vector.tensor_scalar(out=scur[g][:], in0=vb[:],
                                        scalar1=1.0 / MAXQ, scalar2=EPS,
                                        op0=Alu.mult, op1=Alu.max)
                nc.vector.reciprocal(invs[g][:], scur[g][:])
                nc.vector.tensor_scalar(out=va[:], in0=va[:], scalar1=-1.0,
                                        scalar2=None, op0=Alu.mult)  # -xmin
                nc.vector.tensor_mul(vc[:], va[:], invs[g][:])
                nc.vector.tensor_scalar(out=vc[:], in0=vc[:], scalar1=MAGIC,
                                        scalar2=MAGIC, op0=Alu.add,
                                        op1=Alu.subtract)
                nc.vector.tensor_scalar(out=nzp[g][:], in0=vc[:], scalar1=MAXQ,
                                        scalar2=None, op0=Alu.min)  # zp0
                nc.vector.tensor_mul(ocur[g][:], scur[g][:], nzp[g][:])
                nc.vector.tensor_scalar(out=ocur[g][:], in0=ocur[g][:],
                                        scalar1=-1.0, scalar2=None,
                                        op0=Alu.mult)
                comb(sxc[g], vc)
                nc.vector.tensor_scalar(out=sxN[g][:], in0=vc[:],
                                        scalar1=1.0 / float(C), scalar2=None,
                                        op0=Alu.mult)

            # ================= STATS PASS (pipelined) =================
            wq = {}

            def pass_s1(key):
                g, h = chunk(key[2])
                w = wp.tile([128, HF], f32, tag="w", name="w")
                wq[key] = w
                nc.scalar.activation(
                    out=w[:], in_=xsl(g, h), func=Act.Relu,
                    bias=nzp[g][:], scale=invs[g][:])

            def pass_s2(key):
                w = wq[key]
                eng[cfg["ts_b"][key[2]]].tensor_scalar(
                    out=w[:], in0=w[:], scalar1=MAGIC,
                    scalar2=MAGIC + MAXQ, op0=Alu.add, op1=Alu.min)

            def pass_s3(key):
                g, h = chunk(key[2])
                w = wq[key]
                q = qp.tile([128, HF], f16, tag="q", name="q")
                wq[key] = q
                nc.vector.tensor_scalar(
                    out=q[:], in0=w[:], scalar1=MAGIC, scalar2=None,
                    op0=Alu.subtract, op1=Alu.add,
                    accum_out=sqc[g][:, h:h + 1])
                scr16 = sp.tile([128, HF], f16, tag="s16", name="s16")
                nc.vector.scalar_tensor_tensor(
                    out=scr16[:], in0=xsl(g, h), scalar=ocur[g][:],
                    in1=q[:], op0=Alu.subtract, op1=Alu.mult,
                    accum_out=numc[g][:, h:h + 1])

            def pass_s4(key):
                g, h = chunk(key[2])
                q = wq.pop(key)
                if cfg["den"][key[2]] == "A":
                    pscr = psp.tile([128, HF], f32, tag="ps", name="ps")
                    nc.scalar.activation(
                        out=pscr[:], in_=q[:], func=Act.Square,
                        accum_out=denc[g][:, h:h + 1])
                else:
                    scr16b = sp.tile([128, HF], f16, tag="s16b", name="s16b")
                    nc.vector.tensor_tensor_reduce(
                        out=scr16b[:], in0=q[:], in1=q[:], scale=1.0,
                        scalar=0.0, op0=Alu.mult, op1=Alu.add,
                        accum_out=denc[g][:, h:h + 1])

            def upd_group(g):
                pe = nc.gpsimd if cfg.get("upd_pool") else nc.vector
                ua, ub, uc = t1[g], t2[g], t3[g]
                comb(denc[g], ua)
                comb(numc[g], ub)
                comb(sqc[g], uc)
                pe.tensor_scalar(out=ua[:], in0=ua[:], scalar1=EPS,
                                 scalar2=None, op0=Alu.add)
                nc.vector.reciprocal(ua[:], ua[:])
                pe.tensor_tensor(out=ub[:], in0=ub[:], in1=ua[:],
                                 op=Alu.mult)   # cand scale
                pe.tensor_scalar(out=ua[:], in0=ub[:], scalar1=-1.0,
                                 scalar2=EPS, op0=Alu.mult, op1=Alu.max)
                pe.tensor_tensor(out=scur[g][:], in0=ub[:], in1=ua[:],
                                 op=Alu.max)
                pe.tensor_tensor(out=ua[:], in0=scur[g][:], in1=uc[:],
                                 op=Alu.mult)
                nc.vector.scalar_tensor_tensor(
                    out=ocur[g][:], in0=ua[:], scalar=-1.0 / float(C),
                    in1=sxN[g][:], op0=Alu.mult, op1=Alu.add)
                nc.vector.reciprocal(invs[g][:], scur[g][:])
                nc.vector.scalar_tensor_tensor(
                    out=nzp[g][:], in0=ocur[g][:], scalar=-1.0,
                    in1=invs[g][:], op0=Alu.mult, op1=Alu.mult)

            # ================= FINAL PASS (pipelined) =================
            def fin_s1(key):
                g, h = chunk(key[1])
                w = wp.tile([128, HF], f32, tag="w", name="w")
                wq[key] = w
                nc.scalar.activation(
                    out=w[:], in_=xsl(g, h), func=Act.Relu,
                    bias=nzp[g][:], scale=invs[g][:])

            def fin_s2(key):
                w = wq[key]
                eng[cfg["fin_b"][key[1]]].tensor_scalar(
                    out=w[:], in0=w[:], scalar1=MAGIC,
                    scalar2=MAGIC + MAXQ, op0=Alu.add, op1=Alu.min)

            def fin_s3(key):
                w = wq[key]
                q = qp.tile([128, HF], f16, tag="q", name="q")
                wq[key] = q
                eng[cfg["fin_c"][key[1]]].tensor_scalar(
                    out=q[:], in0=w[:], scalar1=MAGIC, scalar2=None,
                    op0=Alu.subtract)

            def fin_s4(key):
                g, h = chunk(key[1])
                q = wq.pop(key)
                rec = wp.tile([128, HF], f32, tag="w", name="w")
                if cfg["fin_r"][key[1]] == "A":
                    nc.scalar.activation(
                        out=rec[:], in_=q[:], func=Act.Identity,
                        bias=ocur[g][:], scale=scur[g][:])
                else:
                    eng[cfg["fin_r"][key[1]]].tensor_scalar(
                        out=rec[:], in0=q[:], scalar1=scur[g][:],
                        scalar2=ocur[g][:], op0=Alu.mult, op1=Alu.add)
                nc.sync.dma_start(
                    out_d[g * 128:(g + 1) * 128, h * HF:(h + 1) * HF],
                    rec[:])

            # ======= EMISSION: one global pipeline across all phases =======
            load_stages = [lambda key: load_s1(key[1]),
                           lambda key: load_s2(key[1]),
                           lambda key: load_s3(key[1]),
                           lambda key: load_s4(key[1])]
            stats_stages = [pass_s1, pass_s2, pass_s3, pass_s4]
            fin_stages = [fin_s1, fin_s2, fin_s3, fin_s4]

            slots = [("load", i) for i in range(NCH)]
            for k in range(N_PASS):
                slots += [("stats", k, i) for i in range(NCH)]
            slots += [("fin", i) for i in range(NCH)]

            pending_hooks = []

            def stage_of(key, s):
                if key[0] == "load":
                    load_stages[s](key)
                    if s == 3 and key[1] % NH == NH - 1:
                        pending_hooks.append((init_group, key[1] // NH))
                elif key[0] == "stats":
                    stats_stages[s](key)
                    if s == 3 and key[2] % NH == NH - 1:
                        pending_hooks.append((upd_group, key[2] // NH))
                else:
                    fin_stages[s](key)

            NS = 4
            for j in range(len(slots) + NS - 1):
                hooks, pending_hooks[:] = pending_hooks[:], []
                for s in range(NS):
                    jj = j - s
                    if 0 <= jj < len(slots):
                        stage_of(slots[jj], s)
                for fn, g in hooks:
                    fn(g)
            for fn, g in pending_hooks:
                fn(g)

    nc.compile()
    return nc


def _get_program():
    if "nc" not in _CACHE:
        _CACHE["nc"] = _build_program()
    return _CACHE["nc"]


def _numpy_fallback(w, mask):
    # Exact mirror of the reference (only used if mask isn't all-ones).
    x = w.astype(np.float32)
    mask_f = mask.astype(np.float32)
    valid = mask_f.sum(axis=1)
    has_valid = valid > 0
    big = np.float32(np.inf)
    xmin = np.where(mask, x, big).min(axis=1)
    xmax = np.where(mask, x, -big).max(axis=1)
    xmin = np.where(has_valid, xmin, -1.0).astype(np.float32)
    xmax = np.where(has_valid, xmax, 1.0).astype(np.float32)
    xmin = np.minimum(xmin, 0.0)
    xmax = np.maximum(xmax, 0.0)
    scale = np.maximum((xmax - xmin) / np.float32(MAXQ), np.float32(EPS))
    zp = np.clip(np.round(-xmin / scale), 0.0, MAXQ).astype(np.float32)
    offset = -scale * zp
    total_valid = max(valid.sum(), 1.0)

    def masked_mse(s, o):
        q = np.clip(np.round((x - o[:, None]) / s[:, None]), 0.0, MAXQ) * mask_f
        recon = s[:, None] * q + o[:, None]
        return (((x - recon) ** 2) * mask_f).sum() / total_valid

    best_mse = masked_mse(scale, offset)
    best_s, best_o = scale.copy(), offset.copy()
    for _ in range(5):
        q = np.clip(np.round((x - offset[:, None]) / scale[:, None]), 0.0, MAXQ) * mask_f
        denom = (q * q).sum(axis=1)
        numer = ((x - offset[:, None]) * q).sum(axis=1)
        new_s = np.where(denom > EPS, numer / (denom + EPS), scale)
        new_s = np.maximum(np.abs(new_s), EPS).astype(np.float32)
        new_o = ((x - new_s[:, None] * q) * mask_f).sum(axis=1) / np.maximum(valid, 1.0)
        scale = np.where(has_valid, new_s, scale).astype(np.float32)
        offset = np.where(has_valid, new_o, offset).astype(np.float32)
        cur = masked_mse(scale, offset)
        if cur < best_mse:
            best_mse, best_s, best_o = cur, scale.copy(), offset.copy()
    scale, offset = best_s, best_o
    q = np.clip(np.round((x - offset[:, None]) / scale[:, None]), 0.0, MAXQ) * mask_f
    x_hat = scale[:, None] * q + offset[:, None]
    return np.where(mask, x_hat, x).astype(np.float32)


def kernel(w, mask):
    w = np.ascontiguousarray(np.asarray(w, dtype=np.float32))
    mask_np = np.asarray(mask)
    if not mask_np.all():
        return _numpy_fallback(w, mask_np)

    from concourse.bass_utils import run_bass_kernel_spmd
    nc = _get_program()
    in_maps = [{"x": np.ascontiguousarray(w[i * R:(i + 1) * R])}
               for i in range(N_CORES)]
    res = run_bass_kernel_spmd(nc, in_maps, list(range(N_CORES))).results
    return np.concatenate([res[i]["out"] for i in range(N_CORES)], axis=0)
